# revision 13
# baseline (speedup 1.0000x reference)
"""Llama4 MoE experts kernel for 8 Trainium2 NeuronCores.

Expert-parallel; tokens are pre-sorted per expert (8192 = 8 experts x 1024),
so core e computes expert e locally:
   out_e = (up_e * silu(gate_e)) @ W2_e,  [gate_e|up_e] = x_e @ W1_e

Phase 1 (gate_up): level-1 Strassen (7 products, halved M/K/N). All
weight-side combos are precomputed on the host; x-side combos are built
on the idle Vector engine while the head DMA streams. SwiGLU is fused
into the combine; inter pieces land in 4 persistent pc tensors at
(f-half x t-half) granularity.

Phase 2 (down): level-2 Strassen (49 products of [512h, 256t], 49/64 of
the PE row-streaming time). Stationary W2 combos come from the host
(51MB streamed on the sync queue - measured single-queue HBM rate is
~390 GB/s; concurrent queues interfere and drop to ~310). Moving
operands are +-sums of <=4 of the 16 inter blocks, built JIT on DVE
from pc slices (1-term operands feed the PE straight from pc). Both
Strassen combine levels are fused: each product accumulates into the
64 [128,256] f32 out tiles with coefficient sigma_m * tau_n, spread
across DVE/Pool with Act doing first-touch copies. Products needing
only f-quarters {0,2} (inner n in {2,6}) run first so their JIT builds
overlap the tail of phase 1.

Phase-1 PE form: psum[f128, t512] += s_m[:,kb,:].T @ b_m[:,kb,:]
Phase-2 PE form: psum[h128, t256] += wt[:,hq,kf,:].T @ mov[:,kf,:]
Output is [h, t]-transposed; host un-transposes.
"""

import numpy as np
import ml_dtypes

E, T, H, F, P = 8, 1024, 2048, 4096, 128
TH = T // 2            # 512 token half
KB1 = 8                # k-blocks per K-half in phase 1 (K=1024)
FB = 32                # 128-wide f blocks per f-half (4096/128)
HB = 8                 # 128-wide h blocks per h-half (1024/128)

# chain order within a phase-1 7-product set (see baseline notes)
CHAIN = [3, 5, 1, 2, 6, 4, 7]
CHAIN0 = [2, 5, 1, 3, 6, 7, 4]

# ---- Strassen (standard convention S) patterns: (row, col, sign) ----
PA = {1: [(0, 0, 1), (1, 1, 1)], 2: [(1, 0, 1), (1, 1, 1)], 3: [(0, 0, 1)],
      4: [(1, 1, 1)], 5: [(0, 0, 1), (0, 1, 1)], 6: [(1, 0, 1), (0, 0, -1)],
      7: [(0, 1, 1), (1, 1, -1)]}
PB = {1: [(0, 0, 1), (1, 1, 1)], 2: [(0, 0, 1)], 3: [(0, 1, 1), (1, 1, -1)],
      4: [(1, 0, 1), (0, 0, -1)], 5: [(1, 1, 1)], 6: [(0, 0, 1), (0, 1, 1)],
      7: [(1, 0, 1), (1, 1, 1)]}
SIG = {1: [(0, 0, 1), (1, 1, 1)], 2: [(1, 0, 1), (1, 1, -1)],
       3: [(0, 1, 1), (1, 1, 1)], 4: [(0, 0, 1), (1, 0, 1)],
       5: [(0, 1, 1), (0, 0, -1)], 6: [(1, 1, 1)], 7: [(0, 0, 1)]}

# phase-2 product order: products whose moving operands only need inter
# f-quarters {0, 2} (inner n in {2, 6}) first - those pieces are complete
# at phase-1's midpoint, so their JIT builds overlap phase-1's tail.
# late group is n-major so the shared t-table entries t(O, n) live only
# within their n-block (lets ttab be a small ring instead of 16 tiles).
P2_ORDER = [(m, n) for m in range(1, 8) for n in (2, 6)] + \
           [(m, n) for n in (1, 3, 4, 5, 7) for m in range(1, 8)]

_CACHE = {}


def _mov_terms(m, n):
    """Signed base16 blocks (a=f-quarter, b=t-quarter, sign) of mov_{m,n}."""
    return [(2 * R + r, 2 * C + c, sg * tg)
            for (R, C, sg) in PB[m] for (r, c, tg) in PB[n]]


def _build():
    import concourse.bacc as bacc
    import concourse.tile as tile
    import concourse.mybir as mybir

    bf16 = mybir.dt.bfloat16
    f32 = mybir.dt.float32
    Silu = mybir.ActivationFunctionType.Silu

    nc = bacc.Bacc("TRN2", target_bir_lowering=False, debug=False, num_devices=E)

    xq_d = nc.dram_tensor("xq", [4, P, KB1 * TH], bf16, kind="ExternalInput").ap()
    w1s_d = nc.dram_tensor("w1s", [7, FB, P, KB1, P], bf16, kind="ExternalInput").ap()
    w2s_d = nc.dram_tensor("w2s", [49, P, 4, 8, P], bf16, kind="ExternalInput").ap()
    out_d = nc.dram_tensor("outT", [2 * HB, P, T], f32, kind="ExternalOutput").ap()

    with tile.TileContext(nc) as tc:
        with (
            tc.tile_pool(name="pc", bufs=1) as pc_pool,         # 4 x [P,16,512] bf16
            tc.tile_pool(name="w2pool", bufs=4) as w2_pool,     # w2 stream ring
            tc.tile_pool(name="movp", bufs=3) as mov_pool,      # JIT moving ring
            tc.tile_pool(name="ps", bufs=8, space="PSUM") as ps_pool,
        ):
            # inter^T pieces: pc[h][s] = (f-half h, t-half s), [P, 16 kf, 512]
            pc00 = pc_pool.tile([P, 16, TH], bf16, name="pc00")
            pc01 = pc_pool.tile([P, 16, TH], bf16, name="pc01")
            pc10 = pc_pool.tile([P, 16, TH], bf16, name="pc10")
            pc11 = pc_pool.tile([P, 16, TH], bf16, name="pc11")
            pc = [[pc00, pc01], [pc10, pc11]]

            def b16(a, b):
                """base16 block (f-quarter a, t-quarter b) as [P, 8, 256] AP."""
                t = pc[a // 2][b // 2]
                ko = (a % 2) * 8
                co = (b % 2) * 256
                return t[:, ko:ko + 8, co:co + 256]

            w2tiles = {}

            def w2_dma(pi, queue=None):
                wt = w2_pool.tile([P, 4, 8, P], bf16, tag="w2", name=f"w2_{pi}")
                (queue or nc.sync).dma_start(out=wt[:], in_=w2s_d[pi])
                w2tiles[pi] = wt

            # ---------------- phase 1 (scoped pools) ----------------
            with (
                tc.tile_pool(name="bmv", bufs=1) as bmv_pool,
                tc.tile_pool(name="spool", bufs=4) as s_pool,
                tc.tile_pool(name="tmp", bufs=8) as tmp_pool,
            ):
                bt = {m: bmv_pool.tile([P, KB1 * TH], bf16, name=f"b{m}")
                      for m in range(1, 8)}
                dx = bmv_pool.tile([P, TH], bf16, name="dx")

                def tmp(nm):
                    return tmp_pool.tile([P, TH], f32, tag="t", name=nm)

                # PE warmup on a zero tile while the head DMA streams
                nc.vector.memset(dx[:], 0)
                for dc in range(4):
                    dpm = ps_pool.tile([P, TH], f32, tag="m", name=f"dummy_{dc}")
                    for kb in range(KB1):
                        nc.tensor.matmul(
                            dpm[:], lhsT=dx[:, 0:P], rhs=dx[:],
                            start=(kb == 0), stop=(kb == KB1 - 1),
                        )

                def s0_dma(m):
                    st = s_pool.tile([P, KB1, P], bf16, tag="s", name=f"s{m}_0")
                    nc.sync.dma_start(out=st[:], in_=w1s_d[m - 1, 0])
                    return st

                HF = KB1 * TH // 2
                s0 = {}
                nc.scalar.dma_start(out=bt[2][:], in_=xq_d[0])      # q11 (= b2)
                nc.sync.dma_start(out=bt[5][:], in_=xq_d[1])        # q22 (= b5)
                s0[2] = s0_dma(2)
                s0[5] = s0_dma(5)
                s0[1] = s0_dma(1)
                nc.scalar.dma_start(out=bt[6][:], in_=xq_d[2])      # q12 (staged)
                s0[3] = s0_dma(3)
                for h in range(2):                                  # b1 = q11+q22
                    hs = slice(h * HF, (h + 1) * HF)
                    nc.vector.tensor_add(bt[1][:, hs], bt[2][:, hs], bt[5][:, hs])
                nc.sync.dma_start(out=bt[4][:], in_=xq_d[3])        # q21 (staged)
                s0[6] = s0_dma(6)
                for h in range(2):                                  # b3 = q12-q22
                    hs = slice(h * HF, (h + 1) * HF)
                    nc.vector.tensor_sub(bt[3][:, hs], bt[6][:, hs], bt[5][:, hs])
                for h in range(2):                                  # b6 = q12+q11
                    hs = slice(h * HF, (h + 1) * HF)
                    nc.vector.tensor_add(bt[6][:, hs], bt[6][:, hs], bt[2][:, hs])
                s0[7] = s0_dma(7)
                for h in range(2):                                  # b7 = q21+q22
                    hs = slice(h * HF, (h + 1) * HF)
                    nc.vector.tensor_add(bt[7][:, hs], bt[4][:, hs], bt[5][:, hs])
                for h in range(2):                                  # b4 = q21-q11
                    hs = slice(h * HF, (h + 1) * HF)
                    nc.vector.tensor_sub(bt[4][:, hs], bt[4][:, hs], bt[2][:, hs])
                s0[4] = s0_dma(4)

                for kf in range(16):
                    for half, fb in ((0, kf), (1, 16 + kf)):
                        ms = {}
                        for m in (CHAIN0 if fb == 0 else CHAIN):
                            if fb == 0:
                                st = s0[m]
                            else:
                                st = s_pool.tile([P, KB1, P], bf16, tag="s",
                                                 name=f"s{m}_{fb}")
                                nc.sync.dma_start(out=st[:], in_=w1s_d[m - 1, fb])
                            pm = ps_pool.tile([P, TH], f32, tag="m",
                                              name=f"p1m{m}_{fb}")
                            for kb in range(KB1):
                                nc.tensor.matmul(
                                    pm[:], lhsT=st[:, kb, :],
                                    rhs=bt[m][:, kb * TH:(kb + 1) * TH],
                                    start=(kb == 0), stop=(kb == KB1 - 1),
                                )
                            ms[m] = pm
                        # combine: C-blocks from M1..M7, SwiGLU fused.
                        a3 = tmp(f"a3_{fb}")
                        nc.scalar.copy(a3[:], ms[3][:])
                        c12 = tmp(f"c12_{fb}")
                        nc.vector.tensor_add(c12[:], a3[:], ms[5][:])
                        sg1 = tmp(f"sg1_{fb}")
                        nc.scalar.activation(sg1[:], c12[:], Silu)
                        a1 = tmp(f"a1_{fb}")
                        nc.scalar.copy(a1[:], ms[1][:])
                        t3 = tmp(f"t3_{fb}")
                        nc.vector.tensor_sub(t3[:], a1[:], ms[2][:])
                        t4 = tmp(f"t4_{fb}")
                        nc.vector.tensor_add(t4[:], a3[:], ms[6][:])
                        c22 = tmp(f"c22_{fb}")
                        nc.gpsimd.tensor_add(c22[:], t3[:], t4[:])
                        # th1 inter piece = silu(C12)*C22 -> pc[half][1]
                        nc.vector.tensor_mul(pc[half][1][:, kf, :], sg1[:], c22[:])
                        # th0 path
                        a4 = tmp(f"a4_{fb}")
                        nc.scalar.copy(a4[:], ms[4][:])
                        c21 = tmp(f"c21_{fb}")
                        nc.vector.tensor_add(c21[:], a4[:], ms[2][:])
                        t1 = tmp(f"t1_{fb}")
                        nc.gpsimd.tensor_add(t1[:], a1[:], a4[:])
                        t2 = tmp(f"t2_{fb}")
                        nc.vector.tensor_sub(t2[:], t1[:], ms[5][:])
                        c11 = tmp(f"c11_{fb}")
                        nc.vector.tensor_add(c11[:], t2[:], ms[7][:])
                        sg0 = tmp(f"sg0_{fb}")
                        nc.scalar.activation(sg0[:], c11[:], Silu)
                        # th0 inter piece = silu(C11)*C21 -> pc[half][0]
                        nc.vector.tensor_mul(pc[half][0][:, kf, :], sg0[:], c21[:])
                    # prefetch the first w2 products once phase-1's own
                    # stream is past its halfway point (scalar queue: the
                    # sync queue's s-tile stream must not be delayed)
                    if kf == 11:
                        for pi in range(3):
                            w2_dma(pi, queue=nc.scalar)

            # ---------------- phase 2 (L2 Strassen) ----------------
            # t-table: shared 2-term inner combos t(O, n) = B[O*n1] + tau2*B[O*n2]
            # (O = outer block position, n = inner 2-term pattern). n=6 entries
            # are consumed by the early product group and built inline; the
            # n in {1,3,4,7} entries live in the post-phase-1 pool.
            with (
                tc.tile_pool(name="ttab", bufs=6) as t_pool,
                tc.tile_pool(name="oacc", bufs=1) as oacc_pool,
            ):
                ttab = {}

                def t_entry(O, n):
                    key = (O, n)
                    if key in ttab:
                        return ttab[key]
                    (r1, c1, t1s), (r2, c2, t2s) = PB[n]
                    assert t1s == 1
                    R, C = O
                    tt = t_pool.tile([P, 8, 256], bf16, tag="tt",
                                     name=f"t_{R}{C}_{n}")
                    op = nc.vector.tensor_add if t2s > 0 else nc.vector.tensor_sub
                    op(tt[:], b16(2 * R + r1, 2 * C + c1), b16(2 * R + r2, 2 * C + c2))
                    ttab[key] = tt
                    return tt

                oacc = {}
                touched = set()
                last_touch = {}
                for idx, (m, n) in enumerate(P2_ORDER):
                    for (Qr, Qc, sg) in SIG[m]:
                        for (qr, qc, tg) in SIG[n]:
                            for pair in range(2):
                                last_touch[(Qr, Qc, qr, qc, pair)] = idx

                # adds: DVE only (GpSimd/Pool cannot read PSUM)
                eng_rr = [nc.vector]
                rr = [0]

                for idx, (m, n) in enumerate(P2_ORDER):
                    pi = idx  # w2s_d is stored in processing order
                    if pi not in w2tiles:
                        w2_dma(pi)
                    if idx + 3 < 49 and (idx + 3) not in w2tiles:
                        w2_dma(idx + 3)
                    wt = w2tiles[pi]

                    # JIT moving operand
                    outerB = PB[m]
                    innerB = PB[n]
                    if len(outerB) == 1 and len(innerB) == 1:
                        (R, C, _), (r, c, _) = outerB[0], innerB[0]
                        mov = b16(2 * R + r, 2 * C + c)
                    elif len(outerB) == 1:        # mov = t(O, n) exactly
                        (R, C, _) = outerB[0]
                        mov = t_entry((R, C), n)
                    elif len(innerB) == 1:        # outer 2-term, single inner pos
                        (R1, C1, _), (R2, C2, s2_) = outerB
                        (r, c, _) = innerB[0]
                        mov = mov_pool.tile([P, 8, 256], bf16, tag="mv",
                                            name=f"mov_{pi}")
                        op = nc.vector.tensor_add if s2_ > 0 else nc.vector.tensor_sub
                        op(mov[:], b16(2 * R1 + r, 2 * C1 + c), b16(2 * R2 + r, 2 * C2 + c))
                    else:                         # 4-term: t(O1,n) +- t(O2,n)
                        (R1, C1, _), (R2, C2, s2_) = outerB
                        ta = t_entry((R1, C1), n)
                        tb = t_entry((R2, C2), n)
                        mov = mov_pool.tile([P, 8, 256], bf16, tag="mv",
                                            name=f"mov_{pi}")
                        op = nc.vector.tensor_add if s2_ > 0 else nc.vector.tensor_sub
                        op(mov[:], ta[:], tb[:])

                    # PE: 4 hq chains of 8 matmuls, packed two per psum tile
                    pm = {}
                    pm[0] = ps_pool.tile([P, TH], f32, tag="m", name=f"p2a_{pi}")
                    pm[1] = ps_pool.tile([P, TH], f32, tag="m", name=f"p2b_{pi}")
                    for hq in range(4):
                        tgt = pm[hq // 2][:, (hq % 2) * 256:(hq % 2 + 1) * 256]
                        for kff in range(8):
                            nc.tensor.matmul(
                                tgt, lhsT=wt[:, hq, kff, :], rhs=mov[:, kff, :],
                                start=(kff == 0), stop=(kff == 7),
                            )

                    # fused combine: whole-psum-tile ops into paired out tiles
                    for (Qr, Qc, sg) in SIG[m]:
                        for (qr, qc, tg) in SIG[n]:
                            coef = sg * tg
                            for pair in range(2):
                                key = (Qr, Qc, qr, qc, pair)
                                if key not in touched:
                                    o = oacc_pool.tile([P, TH], f32,
                                                       name=f"o_{Qr}{Qc}{qr}{qc}{pair}")
                                    oacc[key] = o
                                    if coef == 1:
                                        nc.scalar.copy(o[:], pm[pair][:])
                                    else:
                                        nc.vector.tensor_scalar_mul(o[:], pm[pair][:], -1.0)
                                    touched.add(key)
                                else:
                                    o = oacc[key]
                                    eng = eng_rr[rr[0] % len(eng_rr)]
                                    rr[0] += 1
                                    op = eng.tensor_add if coef > 0 else eng.tensor_sub
                                    op(o[:], o[:], pm[pair][:])
                                if last_touch[key] == idx:
                                    o = oacc[key]
                                    cs = Qc * 512 + qc * 256
                                    for j in range(2):
                                        hb = Qr * 8 + qr * 4 + pair * 2 + j
                                        nc.scalar.dma_start(
                                            out=out_d[hb, :, cs:cs + 256],
                                            in_=o[:, j * 256:(j + 1) * 256])

    nc.compile()
    return nc


def _prep_inputs(hidden_states, gate_up_proj, down_proj):
    bf = ml_dtypes.bfloat16
    xr = np.asarray(hidden_states, np.float32).reshape(E, T, H)
    w1 = np.asarray(gate_up_proj, np.float32)
    w2 = np.asarray(down_proj, np.float32)
    maps = []
    for e in range(E):
        xt = xr[e].T.reshape(H // P, P, T).transpose(1, 0, 2)  # [128, 16, 1024]
        xq = np.stack([
            xt[:, 0:8, 0:TH], xt[:, 8:16, TH:T],
            xt[:, 0:8, TH:T], xt[:, 8:16, 0:TH],
        ]).astype(bf).reshape(4, P, KB1 * TH)  # q11, q22, q12, q21

        W1 = w1[e]
        G1, G2 = W1[0:1024, 0:F], W1[1024:2048, 0:F]
        U1, U2 = W1[0:1024, F:2 * F], W1[1024:2048, F:2 * F]
        s = [G1 + U2, U1 + U2, G1, U2, G1 + G2, U1 - G1, G2 - U2]
        w1s = np.stack([
            sm.reshape(KB1, P, FB, P).transpose(2, 1, 0, 3) for sm in s
        ]).astype(bf)  # [7, 32, 128, 8, 128]

        # phase-2 L2 stationary combos, in processing order.
        # A-role = W2^T; A4[r][c] (h-quarter r, f-quarter c) stored in
        # natural [f, h] orientation = W2[f-quarter c, h-quarter r].
        W2 = w2[e]
        Wblk = lambda c, r: W2[c * 1024:(c + 1) * 1024, r * 512:(r + 1) * 512]
        stats = []
        for (m, n) in P2_ORDER:
            stat = np.zeros((1024, 512), np.float32)
            for (R, C, sg) in PA[m]:
                for (r, c, tg) in PA[n]:
                    stat += sg * tg * Wblk(2 * C + c, 2 * R + r)
            # [1024f, 512h] -> [128 fp, 4 hq, 8 kf, 128 hc]
            stats.append(stat.reshape(8, P, 4, P).transpose(1, 2, 0, 3))
        w2s = np.stack(stats).astype(bf)  # [49, 128, 4, 8, 128]

        maps.append({
            "xq": np.ascontiguousarray(xq),
            "w1s": np.ascontiguousarray(w1s),
            "w2s": np.ascontiguousarray(w2s),
        })
    return maps


def run_spmd(in_maps, trace=False, trace_kwargs=None):
    from concourse.bass_utils import run_bass_kernel_spmd
    from concourse.bass_interp import get_hw_module

    if "nc" not in _CACHE:
        _CACHE["nc"] = _build()
    nc = _CACHE["nc"]

    old_m = nc.m
    nc.m = get_hw_module(nc.m)
    try:
        res = run_bass_kernel_spmd(
            nc, in_maps, core_ids=list(range(E)),
            trace=trace, **(trace_kwargs or {}),
        )
    finally:
        nc.m = old_m
    return res


def kernel(hidden_states, gate_up_proj, down_proj):
    in_maps = _prep_inputs(hidden_states, gate_up_proj, down_proj)
    res = run_spmd(in_maps)
    # outT [16, 128, 1024]: rows h = hb*128+p, cols t
    out = np.concatenate(
        [res.results[e]["outT"].reshape(H, T).T for e in range(E)], axis=0
    )
    return np.ascontiguousarray(out.astype(np.float32))
